# revision 19
# baseline (speedup 1.0000x reference)
"""Trainium2 Bass kernel for nn_BlockLTN (gnn_message_passing).

Math:
    z[o,v,c] = sum_{k,d} x[v,k,d] * W[o,d,k,c] + sum_d b[o,c,d]
    out[e,c,o] = sum_v G[e,v] * z[o,v,c]

Folded:  out[e, c*8+o] = G[e,:] @ Z2[:, c*8+o]
  where  Z2[v, c*8+o] = (x.reshape(V,KD) @ W.transpose(2,1,3,0).reshape(KD,CO))[v, c*8+o]
                        + b.sum(-1).T.reshape(CO)[c*8+o]

The dominant work is the [E,V] @ [V, CO] GEMM over the 256 MB boundary
operator G.  Sharding: G and out row-wise over E across 8 cores (data
parallel over out-simplices); Z2 (8 MB bf16) replicated; no collectives.

Device kernel (per core, E_loc = 1024 rows):
  - Z2 [8192, 512] bf16 resident in SBUF.
  - Stream GT_c = G_c.T [8192, 1024] bf16 in 64 v-chunks of [128, 1024]
    (lhsT layout, pre-transposed on host).
  - Accumulate all 8 PSUM banks: psum[e_tile] += GT_chunk[:, e_tile].T @ Z2_chunk
  - Copy PSUM -> SBUF (fp32) -> HBM.
"""

import numpy as np
import ml_dtypes

V = 8192
E = 8192
K = 64
C = 64
D = 8
O = 8
KD = K * D    # 512
CO = C * O    # 512
N_CORES = 8
EL = E // N_CORES  # 1024 out-rows per core
N_VCHUNK = V // 128  # 64
N_ETILE = EL // 128  # 8

BF16 = ml_dtypes.bfloat16

_cache = {}


def _build_bass():
    import concourse.bass as bass
    import concourse.mybir as mybir
    from concourse import bacc
    from concourse.tile import TileContext

    nc = bacc.Bacc("TRN2", target_bir_lowering=False)

    gt = nc.dram_tensor("gt", (V, EL), mybir.dt.bfloat16, kind="ExternalInput")
    z2 = nc.dram_tensor("z2", (V, CO), mybir.dt.bfloat16, kind="ExternalInput")
    out = nc.dram_tensor("out", (EL, CO), mybir.dt.float32, kind="ExternalOutput")

    # row v = n*128 + p  ->  partition p, chunk n
    gt_r = gt.rearrange("(n p) e -> p n e", p=128)   # [128, 64, 1024]
    z2_r = z2.rearrange("(n p) c -> p n c", p=128)   # [128, 64, 512]

    with TileContext(nc) as tc:
        with (
            tc.tile_pool(name="z2p", bufs=1) as z2p,
            tc.tile_pool(name="gtp", bufs=16) as gtp,
            tc.tile_pool(name="outp", bufs=4) as outp,
            tc.tile_pool(name="ps", bufs=1, space=bass.MemorySpace.PSUM) as psp,
        ):
            psums = [
                psp.tile([128, CO], mybir.dt.float32, name=f"ps{i}", tag=f"ps{i}")
                for i in range(N_ETILE)
            ]

            # Z2 stays fully resident; grouped tiles (distinct tags) so the
            # v=0 matmuls only wait on the first small group, not all 8 MB.
            # All Z2 DMAs free-run on the scalar HWDGE queue; GT streams on
            # sync.  First groups are small so matmuls start ASAP.
            group_sizes = [1, 1, 2, 4] + [4] * 14
            assert sum(group_sizes) == N_VCHUNK
            z2tiles = []  # per v-chunk: (tile, index within tile)
            v0 = 0
            for g, zg in enumerate(group_sizes):
                z2t = z2p.tile(
                    [128, zg, CO], mybir.dt.bfloat16, name=f"z2_{g}", tag=f"z2_{g}"
                )
                nc.scalar.dma_start(z2t[:], z2_r[:, v0:v0 + zg, :])
                for j in range(zg):
                    z2tiles.append((z2t, j))
                v0 += zg

            for v in range(N_VCHUNK):
                gtt = gtp.tile([128, EL], mybir.dt.bfloat16, tag="gtt")
                nc.sync.dma_start(gtt[:], gt_r[:, v, :])
                z2t, j = z2tiles[v]
                for et in range(N_ETILE):
                    nc.tensor.matmul(
                        psums[et][:],
                        lhsT=gtt[:, et * 128:(et + 1) * 128],
                        rhs=z2t[:, j, :],
                        start=(v == 0),
                        stop=(v == N_VCHUNK - 1),
                    )

            for et in range(N_ETILE):
                osb = outp.tile([128, CO], mybir.dt.float32, tag=f"osb{et % 2}")
                # alternate copy engine so the 8 tail evacuations run on
                # VectorE and ScalarE in parallel
                if et % 2 == 0:
                    nc.vector.tensor_copy(osb[:], psums[et][:])
                else:
                    nc.scalar.copy(osb[:], psums[et][:])
                nc.scalar.dma_start(out[et * 128:(et + 1) * 128, :], osb[:])

    nc.compile()
    return nc


def _build_bass_raw():
    """Hand-scheduled variant: same dataflow as the Tile version but with
    manual semaphores, skipping TileContext's startup sem-storm and exit
    drain/barrier butterfly (~16us of fixed overhead)."""
    import concourse.mybir as mybir
    from concourse import bacc

    f32 = mybir.dt.float32
    bf16 = mybir.dt.bfloat16

    nc = bacc.Bacc("TRN2", target_bir_lowering=False)

    gt = nc.dram_tensor("gt", (V, EL), bf16, kind="ExternalInput")
    z2 = nc.dram_tensor("z2", (V, CO), bf16, kind="ExternalInput")
    out = nc.dram_tensor("out", (EL, CO), f32, kind="ExternalOutput")
    gt_r = gt.rearrange("(n p) e -> p n e", p=128)   # [128, 64, 1024]
    z2_r = z2.rearrange("(n p) c -> p n c", p=128)   # [128, 64, 512]

    NSLOT = 16  # gt ring depth (chunks of [128, EL] bf16, 2KB/partition each)
    z2sb = nc.alloc_sbuf_tensor("z2sb", [128, N_VCHUNK, CO], bf16)
    gtsb = nc.alloc_sbuf_tensor("gtsb", [128, NSLOT, EL], bf16)
    osb = nc.alloc_sbuf_tensor("osb", [128, N_ETILE, CO], f32)
    ps = [nc.alloc_psum_tensor(f"ps{i}", [128, CO], f32) for i in range(N_ETILE)]

    s_gt = nc.alloc_semaphore("s_gt")    # gt chunk v landed: 16*(v+1)
    s_z2 = nc.alloc_semaphore("s_z2")    # z2 group g landed: 16*(g+1)
    s_mm = nc.alloc_semaphore("s_mm")    # PE consumed chunk v: v+1
    s_fin = nc.alloc_semaphore("s_fin")  # final (v=63) matmul per e-tile: et+1
    s_cpv = nc.alloc_semaphore("s_cpv")  # DVE psum->sbuf copies done
    s_out = nc.alloc_semaphore("s_out")  # out DMAs landed: 16 each

    # z2 load groups: small first so the v=0 matmuls start ASAP
    group_sizes = [1, 1, 2, 4] + [4] * 14
    assert sum(group_sizes) == N_VCHUNK
    groups = []
    v0 = 0
    for zg in group_sizes:
        groups.append((v0, zg))
        v0 += zg

    all_sems = [s_gt, s_z2, s_mm, s_fin, s_cpv, s_out]

    with nc.Block(name="k", no_gpsimd_drain=True) as blk:

        @blk.sync
        def _(eng):
            for v in range(N_VCHUNK):
                if v >= NSLOT:
                    eng.wait_ge(s_mm, v - NSLOT + 1)
                eng.dma_start(gtsb[:, v % NSLOT, :], gt_r[:, v, :]).then_inc(
                    s_gt, 16
                )
            for k, et in enumerate((0, 2, 4, 6)):
                eng.wait_ge(s_cpv, k + 1)
                eng.dma_start(
                    out[et * 128:(et + 1) * 128, :], osb[:, et, :]
                ).then_inc(s_out, 16)
            eng.wait_ge(s_out, 16 * N_ETILE)
            # leave sems zeroed so a re-execution of the loaded NEFF works
            for s in all_sems:
                eng.sem_clear(s)

        @blk.scalar
        def _(eng):
            for v0g, zg in groups:
                eng.dma_start(
                    z2sb[:, v0g:v0g + zg, :], z2_r[:, v0g:v0g + zg, :]
                ).then_inc(s_z2, 16)
            for et in (1, 3, 5, 7):
                eng.wait_ge(s_fin, et + 1)
                eng.copy(osb[:, et, :], ps[et][:])
                eng.dma_start(
                    out[et * 128:(et + 1) * 128, :], osb[:, et, :]
                ).then_inc(s_out, 16)

        @blk.tensor
        def _(eng):
            landed = 0
            g = 0
            for v in range(N_VCHUNK):
                while v >= landed:
                    landed += groups[g][1]
                    g += 1
                    eng.wait_ge(s_z2, 16 * g)
                eng.wait_ge(s_gt, 16 * (v + 1))
                for et in range(N_ETILE):
                    mm = eng.matmul(
                        ps[et][:],
                        lhsT=gtsb[:, v % NSLOT, et * 128:(et + 1) * 128],
                        rhs=z2sb[:, v, :],
                        start=(v == 0),
                        stop=(v == N_VCHUNK - 1),
                    )
                    if et == N_ETILE - 1 and v < N_VCHUNK - 1:
                        mm.then_inc(s_mm, 1)
                    if v == N_VCHUNK - 1:
                        mm.then_inc(s_fin, 1)

        @blk.vector
        def _(eng):
            for k, et in enumerate((0, 2, 4, 6)):
                eng.wait_ge(s_fin, et + 1)
                eng.tensor_copy(osb[:, et, :], ps[et][:]).then_inc(s_cpv, 1)

    nc.compile()
    return nc


def _prep_inputs(x, G, W, b):
    x = np.asarray(x, dtype=np.float32)
    G = np.asarray(G, dtype=np.float32)
    W = np.asarray(W, dtype=np.float32)
    b = np.asarray(b, dtype=np.float32)

    X2 = np.ascontiguousarray(x.reshape(V, KD))                 # [V, (k,d)]
    WM = np.ascontiguousarray(W.transpose(2, 1, 3, 0).reshape(KD, CO))  # [(k,d),(c,o)]
    bias = b.sum(axis=-1).T.reshape(CO)                          # [(c,o)]
    Z2 = (X2 @ WM + bias[None, :]).astype(BF16)                  # [V, CO]

    GT = G.T.astype(BF16)                                        # [V, E] contiguous
    in_maps = []
    for c in range(N_CORES):
        GTc = np.ascontiguousarray(GT[:, c * EL:(c + 1) * EL])   # [V, EL]
        in_maps.append({"gt": GTc, "z2": Z2})
    return in_maps


IMPL = "raw"  # "raw" (hand-scheduled) or "tile" (TileContext)


def _run(x, G, W, b, trace=False, trace_cores=None):
    import os

    from concourse.bass_utils import run_bass_kernel_spmd

    impl = os.environ.get("KERNEL_IMPL", IMPL)
    if impl not in _cache:
        _cache[impl] = _build_bass_raw() if impl == "raw" else _build_bass()
    nc = _cache[impl]

    in_maps = _prep_inputs(x, G, W, b)
    kw = {}
    if trace_cores is not None:
        kw["trace_cores"] = trace_cores
    res = run_bass_kernel_spmd(
        nc, in_maps, core_ids=list(range(N_CORES)), trace=trace, **kw,
    )
    out = np.concatenate([res.results[c]["out"] for c in range(N_CORES)], axis=0)
    out = out.reshape(E, C, O).astype(np.float32)
    return out, res


def kernel(x, G, W, b):
    out, _ = _run(x, G, W, b, trace=False)
    return out


# revision 20
# speedup vs baseline: 1.0072x; 1.0072x over previous
"""Trainium2 Bass kernel for nn_BlockLTN (gnn_message_passing).

Math:
    z[o,v,c] = sum_{k,d} x[v,k,d] * W[o,d,k,c] + sum_d b[o,c,d]
    out[e,c,o] = sum_v G[e,v] * z[o,v,c]

Folded:  out[e, c*8+o] = G[e,:] @ Z2[:, c*8+o]
  where  Z2[v, c*8+o] = (x.reshape(V,KD) @ W.transpose(2,1,3,0).reshape(KD,CO))[v, c*8+o]
                        + b.sum(-1).T.reshape(CO)[c*8+o]

The dominant work is the [E,V] @ [V, CO] GEMM over the 256 MB boundary
operator G.  Sharding: G and out row-wise over E across 8 cores (data
parallel over out-simplices); Z2 (8 MB bf16) replicated; no collectives.

Device kernel (per core, E_loc = 1024 rows):
  - Z2 [8192, 512] bf16 resident in SBUF.
  - Stream GT_c = G_c.T [8192, 1024] bf16 in 64 v-chunks of [128, 1024]
    (lhsT layout, pre-transposed on host).
  - Accumulate all 8 PSUM banks: psum[e_tile] += GT_chunk[:, e_tile].T @ Z2_chunk
  - Copy PSUM -> SBUF (fp32) -> HBM.
"""

import numpy as np
import ml_dtypes

V = 8192
E = 8192
K = 64
C = 64
D = 8
O = 8
KD = K * D    # 512
CO = C * O    # 512
N_CORES = 8
EL = E // N_CORES  # 1024 out-rows per core
N_VCHUNK = V // 128  # 64
N_ETILE = EL // 128  # 8

BF16 = ml_dtypes.bfloat16

_cache = {}


def _build_bass():
    import concourse.bass as bass
    import concourse.mybir as mybir
    from concourse import bacc
    from concourse.tile import TileContext

    nc = bacc.Bacc("TRN2", target_bir_lowering=False)

    gt = nc.dram_tensor("gt", (V, EL), mybir.dt.bfloat16, kind="ExternalInput")
    z2 = nc.dram_tensor("z2", (V, CO), mybir.dt.bfloat16, kind="ExternalInput")
    out = nc.dram_tensor("out", (EL, CO), mybir.dt.float32, kind="ExternalOutput")

    # row v = n*128 + p  ->  partition p, chunk n
    gt_r = gt.rearrange("(n p) e -> p n e", p=128)   # [128, 64, 1024]
    z2_r = z2.rearrange("(n p) c -> p n c", p=128)   # [128, 64, 512]

    with TileContext(nc) as tc:
        with (
            tc.tile_pool(name="z2p", bufs=1) as z2p,
            tc.tile_pool(name="gtp", bufs=16) as gtp,
            tc.tile_pool(name="outp", bufs=4) as outp,
            tc.tile_pool(name="ps", bufs=1, space=bass.MemorySpace.PSUM) as psp,
        ):
            psums = [
                psp.tile([128, CO], mybir.dt.float32, name=f"ps{i}", tag=f"ps{i}")
                for i in range(N_ETILE)
            ]

            # Z2 stays fully resident; grouped tiles (distinct tags) so the
            # v=0 matmuls only wait on the first small group, not all 8 MB.
            # All Z2 DMAs free-run on the scalar HWDGE queue; GT streams on
            # sync.  First groups are small so matmuls start ASAP.
            group_sizes = [1, 1, 2, 4] + [4] * 14
            assert sum(group_sizes) == N_VCHUNK
            z2tiles = []  # per v-chunk: (tile, index within tile)
            v0 = 0
            for g, zg in enumerate(group_sizes):
                z2t = z2p.tile(
                    [128, zg, CO], mybir.dt.bfloat16, name=f"z2_{g}", tag=f"z2_{g}"
                )
                nc.scalar.dma_start(z2t[:], z2_r[:, v0:v0 + zg, :])
                for j in range(zg):
                    z2tiles.append((z2t, j))
                v0 += zg

            for v in range(N_VCHUNK):
                gtt = gtp.tile([128, EL], mybir.dt.bfloat16, tag="gtt")
                nc.sync.dma_start(gtt[:], gt_r[:, v, :])
                z2t, j = z2tiles[v]
                for et in range(N_ETILE):
                    nc.tensor.matmul(
                        psums[et][:],
                        lhsT=gtt[:, et * 128:(et + 1) * 128],
                        rhs=z2t[:, j, :],
                        start=(v == 0),
                        stop=(v == N_VCHUNK - 1),
                    )

            for et in range(N_ETILE):
                osb = outp.tile([128, CO], mybir.dt.float32, tag=f"osb{et % 2}")
                # alternate copy engine so the 8 tail evacuations run on
                # VectorE and ScalarE in parallel
                if et % 2 == 0:
                    nc.vector.tensor_copy(osb[:], psums[et][:])
                else:
                    nc.scalar.copy(osb[:], psums[et][:])
                nc.scalar.dma_start(out[et * 128:(et + 1) * 128, :], osb[:])

    nc.compile()
    return nc


def _build_bass_raw():
    """Hand-scheduled variant: same dataflow as the Tile version but with
    manual semaphores, skipping TileContext's startup sem-storm and exit
    drain/barrier butterfly (~16us of fixed overhead)."""
    import concourse.mybir as mybir
    from concourse import bacc

    f32 = mybir.dt.float32
    bf16 = mybir.dt.bfloat16

    nc = bacc.Bacc(
        "TRN2",
        target_bir_lowering=False,
        # we never issue SWDGE (gpsimd) DMAs: shrink the DGE scratch-ring
        # init memsets on the critical preamble path, and skip the unused
        # monotonic semaphore setup
        dynamic_dma_scratch_size=2048,
        monotonic_sem_count=0,
    )

    gt = nc.dram_tensor("gt", (V, EL), bf16, kind="ExternalInput")
    z2 = nc.dram_tensor("z2", (V, CO), bf16, kind="ExternalInput")
    out = nc.dram_tensor("out", (EL, CO), f32, kind="ExternalOutput")
    gt_r = gt.rearrange("(n p) e -> p n e", p=128)   # [128, 64, 1024]
    z2_r = z2.rearrange("(n p) c -> p n c", p=128)   # [128, 64, 512]

    NSLOT = 16  # gt ring depth (chunks of [128, EL] bf16, 2KB/partition each)
    z2sb = nc.alloc_sbuf_tensor("z2sb", [128, N_VCHUNK, CO], bf16)
    gtsb = nc.alloc_sbuf_tensor("gtsb", [128, NSLOT, EL], bf16)
    osb = nc.alloc_sbuf_tensor("osb", [128, N_ETILE, CO], f32)
    ps = [nc.alloc_psum_tensor(f"ps{i}", [128, CO], f32) for i in range(N_ETILE)]

    s_gt = nc.alloc_semaphore("s_gt")    # gt chunk v landed: 16*(v+1)
    s_z2 = nc.alloc_semaphore("s_z2")    # z2 group g landed: 16*(g+1)
    s_mm = nc.alloc_semaphore("s_mm")    # PE consumed chunk v: v+1
    s_fin = nc.alloc_semaphore("s_fin")  # final (v=63) matmul per e-tile: et+1
    s_cpv = nc.alloc_semaphore("s_cpv")  # DVE psum->sbuf copies done
    s_out = nc.alloc_semaphore("s_out")  # out DMAs landed: 16 each

    # z2 load groups: small first so the v=0 matmuls start ASAP
    group_sizes = [1, 1, 2, 4] + [4] * 14
    assert sum(group_sizes) == N_VCHUNK
    groups = []
    v0 = 0
    for zg in group_sizes:
        groups.append((v0, zg))
        v0 += zg

    all_sems = [s_gt, s_z2, s_mm, s_fin, s_cpv, s_out]

    with nc.Block(name="k", no_gpsimd_drain=True) as blk:

        @blk.sync
        def _(eng):
            for v in range(N_VCHUNK):
                if v >= NSLOT:
                    eng.wait_ge(s_mm, v - NSLOT + 1)
                eng.dma_start(gtsb[:, v % NSLOT, :], gt_r[:, v, :]).then_inc(
                    s_gt, 16
                )
            for k, et in enumerate((0, 2, 4, 6)):
                eng.wait_ge(s_cpv, k + 1)
                eng.dma_start(
                    out[et * 128:(et + 1) * 128, :], osb[:, et, :]
                ).then_inc(s_out, 16)
            eng.wait_ge(s_out, 16 * N_ETILE)
            # leave sems zeroed so a re-execution of the loaded NEFF works
            for s in all_sems:
                eng.sem_clear(s)

        @blk.scalar
        def _(eng):
            for v0g, zg in groups:
                eng.dma_start(
                    z2sb[:, v0g:v0g + zg, :], z2_r[:, v0g:v0g + zg, :]
                ).then_inc(s_z2, 16)
            for et in (1, 3, 5, 7):
                eng.wait_ge(s_fin, et + 1)
                eng.copy(osb[:, et, :], ps[et][:])
                eng.dma_start(
                    out[et * 128:(et + 1) * 128, :], osb[:, et, :]
                ).then_inc(s_out, 16)

        @blk.tensor
        def _(eng):
            landed = 0
            g = 0
            for v in range(N_VCHUNK):
                while v >= landed:
                    landed += groups[g][1]
                    g += 1
                    eng.wait_ge(s_z2, 16 * g)
                eng.wait_ge(s_gt, 16 * (v + 1))
                for et in range(N_ETILE):
                    mm = eng.matmul(
                        ps[et][:],
                        lhsT=gtsb[:, v % NSLOT, et * 128:(et + 1) * 128],
                        rhs=z2sb[:, v, :],
                        start=(v == 0),
                        stop=(v == N_VCHUNK - 1),
                    )
                    if et == N_ETILE - 1 and v < N_VCHUNK - 1:
                        mm.then_inc(s_mm, 1)
                    if v == N_VCHUNK - 1:
                        mm.then_inc(s_fin, 1)

        @blk.vector
        def _(eng):
            for k, et in enumerate((0, 2, 4, 6)):
                eng.wait_ge(s_fin, et + 1)
                eng.tensor_copy(osb[:, et, :], ps[et][:]).then_inc(s_cpv, 1)

    nc.compile()
    return nc


def _prep_inputs(x, G, W, b):
    x = np.asarray(x, dtype=np.float32)
    G = np.asarray(G, dtype=np.float32)
    W = np.asarray(W, dtype=np.float32)
    b = np.asarray(b, dtype=np.float32)

    X2 = np.ascontiguousarray(x.reshape(V, KD))                 # [V, (k,d)]
    WM = np.ascontiguousarray(W.transpose(2, 1, 3, 0).reshape(KD, CO))  # [(k,d),(c,o)]
    bias = b.sum(axis=-1).T.reshape(CO)                          # [(c,o)]
    Z2 = (X2 @ WM + bias[None, :]).astype(BF16)                  # [V, CO]

    GT = G.T.astype(BF16)                                        # [V, E] contiguous
    in_maps = []
    for c in range(N_CORES):
        GTc = np.ascontiguousarray(GT[:, c * EL:(c + 1) * EL])   # [V, EL]
        in_maps.append({"gt": GTc, "z2": Z2})
    return in_maps


IMPL = "raw"  # "raw" (hand-scheduled) or "tile" (TileContext)


def _run(x, G, W, b, trace=False, trace_cores=None):
    import os

    from concourse.bass_utils import run_bass_kernel_spmd

    impl = os.environ.get("KERNEL_IMPL", IMPL)
    if impl not in _cache:
        _cache[impl] = _build_bass_raw() if impl == "raw" else _build_bass()
    nc = _cache[impl]

    in_maps = _prep_inputs(x, G, W, b)
    kw = {}
    if trace_cores is not None:
        kw["trace_cores"] = trace_cores
    res = run_bass_kernel_spmd(
        nc, in_maps, core_ids=list(range(N_CORES)), trace=trace, **kw,
    )
    out = np.concatenate([res.results[c]["out"] for c in range(N_CORES)], axis=0)
    out = out.reshape(E, C, O).astype(np.float32)
    return out, res


def kernel(x, G, W, b):
    out, _ = _run(x, G, W, b, trace=False)
    return out
